# revision 10
# baseline (speedup 1.0000x reference)
"""Trainium2 Bass kernel for nn_DataReuploadingTorso (8-qubit data-reuploading
quantum circuit, batch 16384).

Math: each PennyLane Rot(phi,theta,omega) = RZ(omega+pi/2) H RZ(theta) H RZ(phi-pi/2).
The whole circuit therefore becomes 17 batch-dependent diagonal phase layers
(phases are a linear map of the observation -> one small matmul + sin/cos)
interleaved with 17 applications of the fixed H^{x8} matrix (TensorE matmuls
with a +-1/16 stationary), plus |.|^2 and a fixed Z-projection at the end.

Sharding: pure data-parallel over batch across 8 NeuronCores (2048 each).

State layout on device (per 256-batch chunk): transposed - state index s on
partitions, batch on free dim. Packed tiles (128, 512) = [s in [0,128) x 256b |
s in [128,256) x 256b].
"""
import numpy as np

import concourse.bass as bass
import concourse.mybir as mybir
import concourse.tile as tile
from concourse.bass_utils import run_bass_kernel_spmd

N_CORES = 8
B_TOTAL = 16384
B_CORE = B_TOTAL // N_CORES      # 2048
F = 256                          # batch per chunk
NCH = B_CORE // F                # 8 chunks
NSTEP = 17
DIM = 256
N_Q = 8

F32 = mybir.dt.float32
F32R = mybir.dt.float32r
I32 = mybir.dt.int32
AOT = mybir.AluOpType
ACTF = mybir.ActivationFunctionType

PI = float(np.pi)
INV_2PI = float(np.float32(1.0 / (2.0 * np.pi)))
K_SIN = 32.5          # q_sin = phi/2pi + 32.5  (phi == 2pi q - 65pi == 2pi q - pi mod 2pi)
NEG_2PI = float(np.float32(-2.0 * np.pi))
TWO_PI = float(np.float32(2.0 * np.pi))


# ----------------------------------------------------------------- host tables
def _build_host_tables(theta, omega):
    """W_aug (17, 13, 256) with Phi_k[b,s] = [x,1][b] @ W_aug[k]; M = H^{x7}/16
    sign matrix (128,128); Z (256,8) PauliZ table."""
    theta = np.asarray(theta, np.float64)              # (8, 5, 3)
    omega = np.asarray(omega, np.float64).reshape(5, 8, 3)

    idx = np.arange(DIM)
    beta = np.stack([(idx >> (7 - q)) & 1 for q in range(N_Q)], 0)   # (8, 256)
    sgn = (2 * beta - 1).astype(np.float64)

    def czterm(pairs):
        t = np.zeros(DIM)
        for a, b in pairs:
            t += np.pi * (beta[a] * beta[b])
        return t
    cz_even = czterm([(0, 1), (2, 3), (4, 5), (6, 7)])
    cz_odd = czterm([(1, 2), (3, 4), (5, 6)])

    steps = []
    for l in range(4):
        A = np.zeros((12, 8)); k = np.zeros(8)
        for q in range(8):
            A[3 * (q % 4) + 1, q] = omega[l, q, 1]
        steps.append((A, k, None))
        A = np.zeros((12, 8)); k = np.zeros(8)
        for q in range(8):
            A[3 * (q % 4) + 2, q] = omega[l, q, 2]
            k[q] = theta[q, l, 0]
        steps.append((A, k, None))
        A = np.zeros((12, 8)); k = theta[:, l, 1].copy()
        steps.append((A, k, None))
        A = np.zeros((12, 8)); k = theta[:, l, 2].copy()
        for q in range(8):
            A[3 * (q % 4) + 0, q] = omega[l + 1, q, 0]
        if l + 1 == 4:
            k += theta[:, 4, 0]
        steps.append((A, k, cz_even if l % 2 == 0 else cz_odd))
    A = np.zeros((12, 8)); k = theta[:, 4, 1].copy()
    for q in range(8):
        A[3 * (q % 4) + 1, q] = omega[4, q, 1]
    steps.append((A, k, None))

    W_aug = np.zeros((NSTEP, 13, DIM))
    for i, (A, k, cz) in enumerate(steps):
        W_aug[i, :12] = A @ (sgn * 0.5)
        c = k @ (sgn * 0.5)
        if cz is not None:
            c = c + cz
        W_aug[i, 12] = np.mod(c + np.pi, 2 * np.pi) - np.pi

    sp = np.arange(DIM)
    pop = np.zeros((DIM, DIM), np.int64)
    for q in range(8):
        pop += np.outer((sp >> q) & 1, (sp >> q) & 1)
    H256sgn = np.where(pop % 2 == 0, 1.0, -1.0)
    M = (H256sgn[:128, :128] / 16.0).astype(np.float32)

    Z = np.stack([1.0 - 2.0 * ((idx >> (7 - q)) & 1) for q in range(8)], 1)
    return W_aug.astype(np.float32), M, Z.astype(np.float32)


# -------------------------------------------------------------- device program
def _legalize_waits(nc, limit=1):
    """walrus codegen allows only one embedded sync-wait on several TRN2
    instruction encodings (notably self-loading fp32/fp32r Matmult). Hoist
    excess waits emitted by Tile onto same-engine NoOps placed just before."""
    def fix_block(blk):
        new_insts = []
        for ins in blk.instructions:
            si = getattr(ins, "sync_info", None)
            waits = list(si.on_wait) if si and si.on_wait else []
            if len(waits) > limit:
                keep = waits[-limit:]
                for j, w in enumerate(waits[:-limit]):
                    new_insts.append(mybir.InstNoOp(
                        name=f"{ins.name}-w{j}",
                        engine=ins.engine,
                        sync_info=mybir.SyncInfo(on_wait=[w], on_update=[]),
                    ))
                si.on_wait = keep
            new_insts.append(ins)
        blk.instructions = new_insts
        for sb in getattr(blk, "blocks", None) or []:
            fix_block(sb)
    for f in nc.m.functions:
        for blk in f.blocks:
            fix_block(blk)


def _build_program():
    nc = bass.Bass("TRN2", target_bir_lowering=False, debug=False,
                   enable_asserts=False, num_devices=N_CORES)

    xT_d = nc.dram_tensor("xT", [13, B_CORE], F32, kind="ExternalInput")
    W_d = nc.dram_tensor("W", [13, NSTEP * DIM], F32, kind="ExternalInput")
    Mp_d = nc.dram_tensor("Mp", [128, 128], F32, kind="ExternalInput")
    Mn_d = nc.dram_tensor("Mn", [128, 128], F32, kind="ExternalInput")
    Z_d = nc.dram_tensor("Zt", [DIM, 8], F32, kind="ExternalInput")
    out_d = nc.dram_tensor("out", [B_CORE, 8], F32, kind="ExternalOutput")

    with tile.TileContext(nc) as tc:
        with (
            tc.tile_pool(name="consts", bufs=1) as consts,
            tc.tile_pool(name="phi", bufs=2, space="PSUM") as phi_pool,
            tc.tile_pool(name="hpsum", bufs=4, space="PSUM") as hpsum_pool,
            tc.tile_pool(name="zpsum", bufs=2, space="PSUM") as zpsum_pool,
            tc.tile_pool(name="mods", bufs=4) as mod_pool,
            tc.tile_pool(name="cs", bufs=4) as cs_pool,
            tc.tile_pool(name="state", bufs=4) as state_pool,
            tc.tile_pool(name="prods", bufs=8) as prod_pool,
            tc.tile_pool(name="tail", bufs=4) as tail_pool,
        ):
            xT = consts.tile([13, B_CORE], F32, tag="xT")
            nc.sync.dma_start(xT[:], xT_d[:])
            Wt = consts.tile([13, NSTEP * DIM], F32, tag="W")
            nc.sync.dma_start(Wt[:], W_d[:])
            Mp = consts.tile([128, 128], F32R, tag="Mp")
            nc.sync.dma_start(Mp[:], Mp_d[:].bitcast(F32R))
            Mn = consts.tile([128, 128], F32R, tag="Mn")
            nc.sync.dma_start(Mn[:], Mn_d[:].bitcast(F32R))
            Z0 = consts.tile([128, 8], F32, tag="Z0")
            nc.sync.dma_start(Z0[:], Z_d[0:128, :])
            Z1 = consts.tile([128, 8], F32, tag="Z1")
            nc.sync.dma_start(Z1[:], Z_d[128:256, :])

            def h_mms(dst_psum, src_tile):
                # dst[h'] = sum_h (+-M) @ src[h]   (H^{x8} block structure)
                for hp in (0, 1):
                    for h in (0, 1):
                        lhsT = Mn if (hp == 1 and h == 1) else Mp
                        nc.tensor.matmul(
                            dst_psum[:, hp * F:(hp + 1) * F],
                            lhsT[:],
                            src_tile[:, h * F:(h + 1) * F],
                            start=(h == 0), stop=(h == 1),
                        )

            for ch in range(NCH):
                bsl = slice(ch * F, (ch + 1) * F)
                st_re = st_im = None
                for k in range(NSTEP):
                    # Phi_k = W_k^T @ x  -> psum (128, 512) packed halves
                    phi = phi_pool.tile([128, 2 * F], F32, tag="phi")
                    for h in (0, 1):
                        nc.tensor.matmul(
                            phi[:, h * F:(h + 1) * F],
                            Wt[:, k * DIM + h * 128: k * DIM + (h + 1) * 128],
                            xT[:, bsl],
                            start=True, stop=True,
                        )
                    # range reduction: q = phi/2pi + K; f = q - rne(q) in
                    # [-.5,.5]; sin(phi) = Sin(-2pi f) (table domain [-pi,pi])
                    qs = mod_pool.tile([128, 2 * F], F32, tag="mod")
                    nc.vector.tensor_scalar(qs[:], phi[:], INV_2PI, K_SIN,
                                            AOT.mult, AOT.add)
                    qc = mod_pool.tile([128, 2 * F], F32, tag="mod")
                    nc.vector.tensor_scalar(qc[:], qs[:], 0.25, None, AOT.add)
                    qsi = mod_pool.tile([128, 2 * F], I32, tag="modi")
                    nc.vector.tensor_copy(qsi[:], qs[:])
                    qci = mod_pool.tile([128, 2 * F], I32, tag="modi")
                    nc.vector.tensor_copy(qci[:], qc[:])
                    msin = mod_pool.tile([128, 2 * F], F32, tag="mod")
                    nc.vector.scalar_tensor_tensor(msin[:], qsi[:], -1.0, qs[:],
                                                   AOT.mult, AOT.add)
                    mcos = mod_pool.tile([128, 2 * F], F32, tag="mod")
                    nc.vector.scalar_tensor_tensor(mcos[:], qci[:], -1.0, qc[:],
                                                   AOT.mult, AOT.add)
                    if k == 0:
                        # state_1 = e^{i Phi_1} (uniform H|0> folded into scale)
                        st_re = state_pool.tile([128, 2 * F], F32R, tag="state")
                        nc.scalar.activation(st_re[:], mcos[:], ACTF.Sin, scale=NEG_2PI)
                        st_im = state_pool.tile([128, 2 * F], F32R, tag="state")
                        nc.scalar.activation(st_im[:], msin[:], ACTF.Sin, scale=NEG_2PI)
                        continue
                    C = cs_pool.tile([128, 2 * F], F32, tag="cs")
                    nc.scalar.activation(C[:], mcos[:], ACTF.Sin, scale=NEG_2PI)
                    S = cs_pool.tile([128, 2 * F], F32, tag="cs")
                    nc.scalar.activation(S[:], msin[:], ACTF.Sin, scale=NEG_2PI)

                    pre = hpsum_pool.tile([128, 2 * F], F32, tag="hpsum")
                    h_mms(pre, st_re)
                    pim = hpsum_pool.tile([128, 2 * F], F32, tag="hpsum")
                    h_mms(pim, st_im)

                    # D_k: (re + i im) *= C + i S
                    p_rc = prod_pool.tile([128, 2 * F], F32, tag="prod")
                    nc.vector.tensor_mul(p_rc[:], pre[:], C[:])
                    p_is = prod_pool.tile([128, 2 * F], F32, tag="prod")
                    nc.vector.tensor_mul(p_is[:], pim[:], S[:])
                    p_rs = prod_pool.tile([128, 2 * F], F32, tag="prod")
                    nc.vector.tensor_mul(p_rs[:], pre[:], S[:])
                    p_ic = prod_pool.tile([128, 2 * F], F32, tag="prod")
                    nc.vector.tensor_mul(p_ic[:], pim[:], C[:])
                    st_re = state_pool.tile([128, 2 * F], F32R, tag="state")
                    nc.gpsimd.tensor_sub(st_re[:], p_rc[:], p_is[:])
                    st_im = state_pool.tile([128, 2 * F], F32R, tag="state")
                    nc.gpsimd.tensor_add(st_im[:], p_rs[:], p_ic[:])

                # final H, probs, Z-projection
                pre = hpsum_pool.tile([128, 2 * F], F32, tag="hpsum")
                h_mms(pre, st_re)
                pim = hpsum_pool.tile([128, 2 * F], F32, tag="hpsum")
                h_mms(pim, st_im)
                p1 = tail_pool.tile([128, 2 * F], F32, tag="tail")
                nc.scalar.activation(p1[:], pre[:], ACTF.Square, scale=1.0 / 16.0)
                p2 = tail_pool.tile([128, 2 * F], F32, tag="tail")
                nc.scalar.activation(p2[:], pim[:], ACTF.Square, scale=1.0 / 16.0)
                probs = tail_pool.tile([128, 2 * F], F32, tag="tail")
                nc.gpsimd.tensor_add(probs[:], p1[:], p2[:])
                for sub in (0, 1):
                    zp = zpsum_pool.tile([128, 8], F32, tag="zp")
                    nc.tensor.matmul(zp[:], probs[:, 0 * F + sub * 128: 0 * F + (sub + 1) * 128],
                                     Z0[:], start=True, stop=False)
                    nc.tensor.matmul(zp[:], probs[:, 1 * F + sub * 128: 1 * F + (sub + 1) * 128],
                                     Z1[:], start=False, stop=True)
                    zs = tail_pool.tile([128, 8], F32, tag="zs")
                    nc.scalar.activation(zs[:], zp[:], ACTF.Copy)
                    nc.sync.dma_start(out_d[ch * F + sub * 128: ch * F + (sub + 1) * 128, :],
                                      zs[:])
    _legalize_waits(nc)
    return nc


_PROGRAM_CACHE = {}


def kernel(observation, theta, omega, _trace=False):
    observation = np.asarray(observation, np.float32)
    W_aug, M, Z = _build_host_tables(theta, omega)
    W_flat = np.ascontiguousarray(
        W_aug.transpose(1, 0, 2).reshape(13, NSTEP * DIM))
    x_augT = np.concatenate(
        [observation, np.ones((B_TOTAL, 1), np.float32)], 1).T  # (13, 16384)
    x_augT = np.ascontiguousarray(x_augT)

    if "nc" not in _PROGRAM_CACHE:
        _PROGRAM_CACHE["nc"] = _build_program()
    nc = _PROGRAM_CACHE["nc"]

    in_maps = []
    for c in range(N_CORES):
        in_maps.append({
            "xT": np.ascontiguousarray(x_augT[:, c * B_CORE:(c + 1) * B_CORE]),
            "W": W_flat,
            "Mp": M,
            "Mn": np.ascontiguousarray(-M),
            "Zt": Z,
        })
    res = run_bass_kernel_spmd(nc, in_maps, core_ids=list(range(N_CORES)),
                               trace=_trace)
    out = np.concatenate([r["out"] for r in res.results], 0)
    if _trace:
        kernel.last_results = res
    return out
